# revision 2
# baseline (speedup 1.0000x reference)
"""Trainium2 Bass kernel for nn_MultiHeadAttention_53815940219243.

Reference computation (single-head attention with full 1024-dim contraction):
    q = x @ Wq + bq; k = x @ Wk + bk; v = x @ Wv + bv        # [4096, 1024]
    scores = softmax(q @ k.T, axis=-1) / sqrt(64)            # [4096, 4096]
    z = scores @ v                                           # [4096, 1024]
    out = z @ Wo + bo                                        # [4096, 64]

Sharding: sequence dim of Q/scores/output split across 8 cores (512 rows each).
K and V are computed sharded (each core does its own 512 rows) and exchanged
with two AllGather collectives, overlapped with the Q projection.

Dataflow trick: everything is computed in "transposed score space" so that no
on-device transposes are needed:
    QT = Wq^T @ xT            (lhsT = Wq as stored, rhs = xT)     [1024, 512]
    KT = Wk^T @ xT                                                [1024, 512/core]
    v  = xT^T @ Wv            (lhsT = xT, rhs = Wv, natural)      [512/core, 1024]
    ST = K @ QT               (lhsT = KT chunk, rhs = QT chunk)   [4096, 512]
    ET = exp(ST)              (no max subtraction: |S| <~ 60, safe in fp32 range)
    den = ones^T @ ET         (column sums, PSUM accumulation)    [1, 512]
    UT = v^T-contracted: lhsT = v chunk, rhs = ET chunk           [1024, 512]
    zT = UT * (1/(8*den)) broadcast
    out = zT^T @ Wo + bo      (lhsT = zT chunk, rhs = Wo)         [512, 64]

Numerics: matmul operands use the TRN2 fp32r dtype (fp32 storage, ~13-bit
mantissa in the PE, 1 cycle/row for N>=256) measured at ~1.5e-4 relative error
per matmul; the final Wo matmul runs in full fp32.
"""

import numpy as np

N = 4096
D = 1024
H = 64
NCORES = 8
NSH = N // NCORES  # 512 rows per core
P = 128
DT = D // P  # 8 partition tiles over the 1024 feature dim
JT = N // P  # 32 partition tiles over the full sequence
JSH = NSH // P  # 4 sequence tiles per core shard
IT = NSH // P  # 4 output row tiles per core

_CACHE = {}


def _build():
    import concourse.bass as bass
    import concourse.mybir as mybir
    import concourse.tile as tile
    from concourse import bacc
    from contextlib import ExitStack

    f32 = mybir.dt.float32
    f32r = mybir.dt.float32r

    nc = bacc.Bacc("TRN2", target_bir_lowering=False, num_devices=NCORES)

    # ---- kernel I/O (per core) ----
    xcT = nc.dram_tensor("xcT", [D, NSH], f32r, kind="ExternalInput")
    wq = nc.dram_tensor("wq", [D, D], f32r, kind="ExternalInput")
    wk = nc.dram_tensor("wk", [D, D], f32r, kind="ExternalInput")
    wv = nc.dram_tensor("wv", [D, D], f32r, kind="ExternalInput")
    wo = nc.dram_tensor("wo", [D, H], f32, kind="ExternalInput")
    bqr = nc.dram_tensor("bqr", [1, D], f32r, kind="ExternalInput")
    bkr = nc.dram_tensor("bkr", [1, D], f32r, kind="ExternalInput")
    bvr = nc.dram_tensor("bvr", [1, D], f32r, kind="ExternalInput")
    bor = nc.dram_tensor("bor", [1, H], f32, kind="ExternalInput")
    ones_row = nc.dram_tensor("ones_row", [1, NSH], f32r, kind="ExternalInput")
    ones_col = nc.dram_tensor("ones_col", [P, 1], f32r, kind="ExternalInput")
    eighth_row = nc.dram_tensor("eighth_row", [1, P], f32r, kind="ExternalInput")
    ones32_row = nc.dram_tensor("ones32_row", [1, P], f32, kind="ExternalInput")

    out = nc.dram_tensor("out", [NSH, H], f32, kind="ExternalOutput")

    # ---- internal DRAM for the collectives ----
    ag_in_kt = nc.dram_tensor("ag_in_kt", [D, NSH], f32r)
    ag_out_kt = nc.dram_tensor("ag_out_kt", [NCORES, D, NSH], f32r)
    ag_in_v = nc.dram_tensor("ag_in_v", [NSH, D], f32r)
    ag_out_v = nc.dram_tensor("ag_out_v", [NCORES, NSH, D], f32r)

    with tile.TileContext(nc) as tc, ExitStack() as ctx:
        # persistent pools
        persist = ctx.enter_context(tc.tile_pool(name="persist", bufs=1))
        small = ctx.enter_context(tc.tile_pool(name="small", bufs=1))

        # resident tiles
        qt_sb = persist.tile([P, DT, NSH], f32r, tag="qt")       # Q^T  16KB/part
        et_sb = persist.tile([P, JT, NSH], f32r, tag="et")       # E^T  64KB/part
        zt_sb = persist.tile([P, DT, NSH], f32, tag="zt")        # z^T  16KB/part
        sbc_sb = persist.tile([P, NSH], f32, tag="sbc")          # 1/(8 den) bcast

        # small constant tiles
        ones_row_sb = small.tile([1, NSH], f32r, tag="onesr")
        ones_col_sb = small.tile([P, 1], f32r, tag="onesc")
        eighth_sb = small.tile([1, P], f32r, tag="eighth")
        ones32_sb = small.tile([1, P], f32, tag="ones32")
        bq_sb = small.tile([1, D], f32r, tag="bq")
        bk_sb = small.tile([1, D], f32r, tag="bk")
        bv_sb = small.tile([1, D], f32r, tag="bv")
        bo_sb = small.tile([1, H], f32, tag="bo")
        wo_sb = small.tile([P, DT, H], f32, tag="wo")
        den_row = small.tile([1, NSH], f32, tag="denr")
        den_row_r = small.tile([1, NSH], f32r, tag="denrr")

        nc.sync.dma_start(out=ones_row_sb[:], in_=ones_row[:, :])
        nc.sync.dma_start(out=ones_col_sb[:], in_=ones_col[:, :])
        nc.sync.dma_start(out=eighth_sb[:], in_=eighth_row[:, :])
        nc.sync.dma_start(out=ones32_sb[:], in_=ones32_row[:, :])
        nc.sync.dma_start(out=bq_sb[:], in_=bqr[:, :])
        nc.sync.dma_start(out=bk_sb[:], in_=bkr[:, :])
        nc.sync.dma_start(out=bv_sb[:], in_=bvr[:, :])
        nc.sync.dma_start(out=bo_sb[:], in_=bor[:, :])
        nc.sync.dma_start(out=wo_sb[:], in_=wo[:, :].rearrange("(t p) h -> p t h", p=P))

        # ---------------- phase A: projections ----------------
        with (
            tc.tile_pool(name="pa_x", bufs=1) as pax,
            tc.tile_pool(name="pa_sbuf", bufs=3) as pa,
            tc.tile_pool(name="pa_w", bufs=2) as paw,
            tc.tile_pool(name="pa_psum", bufs=3, space="PSUM") as pap,
        ):
            xcT_sb = pax.tile([P, DT, NSH], f32r, tag="xct")
            nc.sync.dma_start(
                out=xcT_sb[:], in_=xcT[:, :].rearrange("(t p) i -> p t i", p=P)
            )

            # K^T shard: KT[:, c] = Wk^T @ xT + bk  -> ag_in_kt
            for t in range(DT):
                w_t = paw.tile([P, DT, P], f32r, tag="wqk")
                nc.sync.dma_start(
                    out=w_t[:],
                    in_=wk[:, :].rearrange("(dt p) n -> p dt n", p=P)[
                        :, :, t * P : (t + 1) * P
                    ],
                )
                ps = pap.tile([P, NSH], mybir.dt.float32, tag="pa")
                for dt_i in range(DT):
                    nc.tensor.matmul(
                        ps[:], w_t[:, dt_i, :], xcT_sb[:, dt_i, :],
                        start=(dt_i == 0), stop=False,
                    )
                nc.tensor.matmul(
                    ps[:], bk_sb[0:1, t * P : (t + 1) * P], ones_row_sb[0:1, :],
                    start=False, stop=True,
                )
                kt_t = pa.tile([P, NSH], f32r, tag="ktsh")
                nc.vector.tensor_copy(out=kt_t[:], in_=ps[:])
                nc.sync.dma_start(out=ag_in_kt[t * P : (t + 1) * P, :], in_=kt_t[:])

            nc.gpsimd.collective_compute(
                "AllGather",
                mybir.AluOpType.bypass,
                replica_groups=[list(range(NCORES))],
                ins=[ag_in_kt[:, :].opt()],
                outs=[ag_out_kt[:, :, :].opt()],
            )

            # v shard: v = x_c @ Wv + bv  (natural layout) -> ag_in_v
            for b in range(2):
                wv_b = paw.tile([P, DT, 512], f32r, tag="wv")
                nc.sync.dma_start(
                    out=wv_b[:],
                    in_=wv[:, :].rearrange("(dt p) n -> p dt n", p=P)[
                        :, :, b * 512 : (b + 1) * 512
                    ],
                )
                for j in range(JSH):
                    ps = pap.tile([P, 512], mybir.dt.float32, tag="pa")
                    for dt_i in range(DT):
                        nc.tensor.matmul(
                            ps[:],
                            xcT_sb[:, dt_i, j * P : (j + 1) * P],
                            wv_b[:, dt_i, :],
                            start=(dt_i == 0), stop=False,
                        )
                    nc.tensor.matmul(
                        ps[:], ones_row_sb[0:1, 0:P],
                        bv_sb[0:1, b * 512 : (b + 1) * 512],
                        start=False, stop=True,
                    )
                    v_t = pa.tile([P, 512], f32r, tag="vsh")
                    nc.vector.tensor_copy(out=v_t[:], in_=ps[:])
                    nc.sync.dma_start(
                        out=ag_in_v[j * P : (j + 1) * P, b * 512 : (b + 1) * 512],
                        in_=v_t[:],
                    )

            nc.gpsimd.collective_compute(
                "AllGather",
                mybir.AluOpType.bypass,
                replica_groups=[list(range(NCORES))],
                ins=[ag_in_v[:, :].opt()],
                outs=[ag_out_v[:, :, :].opt()],
            )

            # Q^T: QT = Wq^T @ xT + bq  (resident)
            for t in range(DT):
                w_t = paw.tile([P, DT, P], f32r, tag="wqk")
                nc.sync.dma_start(
                    out=w_t[:],
                    in_=wq[:, :].rearrange("(dt p) n -> p dt n", p=P)[
                        :, :, t * P : (t + 1) * P
                    ],
                )
                ps = pap.tile([P, NSH], mybir.dt.float32, tag="pa")
                for dt_i in range(DT):
                    nc.tensor.matmul(
                        ps[:], w_t[:, dt_i, :], xcT_sb[:, dt_i, :],
                        start=(dt_i == 0), stop=False,
                    )
                nc.tensor.matmul(
                    ps[:], bq_sb[0:1, t * P : (t + 1) * P], ones_row_sb[0:1, :],
                    start=False, stop=True,
                )
                nc.vector.tensor_copy(out=qt_sb[:, t, :], in_=ps[:])

        # ---------------- phase S: scores + exp + denominator ----------------
        with (
            tc.tile_pool(name="ps_kt", bufs=2) as pskt,
            tc.tile_pool(name="ps_psum", bufs=3, space="PSUM") as psp,
            tc.tile_pool(name="ps_den", bufs=1, space="PSUM") as psd,
        ):
            den_ps = psd.tile([1, NSH], mybir.dt.float32, tag="den")
            for r in range(NCORES):
                kt_r = pskt.tile([P, DT, NSH], f32r, tag="ktr")
                nc.sync.dma_start(
                    out=kt_r[:],
                    in_=ag_out_kt[r, :, :].rearrange("(t p) j -> p t j", p=P),
                )
                for jj in range(JSH):
                    jt = r * JSH + jj
                    ps = psp.tile([P, NSH], mybir.dt.float32, tag="st")
                    for dt_i in range(DT):
                        nc.tensor.matmul(
                            ps[:],
                            kt_r[:, dt_i, jj * P : (jj + 1) * P],
                            qt_sb[:, dt_i, :],
                            start=(dt_i == 0), stop=(dt_i == DT - 1),
                        )
                    nc.scalar.activation(
                        out=et_sb[:, jt, :], in_=ps[:],
                        func=mybir.ActivationFunctionType.Exp,
                    )
                    nc.tensor.matmul(
                        den_ps[:], ones_col_sb[:, 0:1], et_sb[:, jt, :],
                        start=(jt == 0), stop=(jt == JT - 1),
                    )

            # s = 1/(8*den) broadcast to 128 partitions
            nc.vector.reciprocal(out=den_row[:], in_=den_ps[:])
            nc.vector.tensor_copy(out=den_row_r[:], in_=den_row[:])
            bc_ps = psp.tile([P, NSH], mybir.dt.float32, tag="bc")
            nc.tensor.matmul(
                bc_ps[:], eighth_sb[0:1, :], den_row_r[0:1, :], start=True, stop=True
            )
            nc.vector.tensor_copy(out=sbc_sb[:], in_=bc_ps[:])

        # ---------------- phase U: z^T = (v^T-contraction of E^T) * s ----------------
        with (
            tc.tile_pool(name="pu_v", bufs=2) as puv,
            tc.tile_pool(name="pu_psum", bufs=2, space="PSUM") as pup,
        ):
            v_re = ag_out_v[:, :, :].rearrange("r (q p) d -> p (r q) d", p=P)
            for dt_i in range(DT):
                v_dt = puv.tile([P, JT, P], f32r, tag="vdt")
                nc.sync.dma_start(
                    out=v_dt[:], in_=v_re[:, :, dt_i * P : (dt_i + 1) * P]
                )
                ps = pup.tile([P, NSH], mybir.dt.float32, tag="ut")
                for jt in range(JT):
                    nc.tensor.matmul(
                        ps[:], v_dt[:, jt, :], et_sb[:, jt, :],
                        start=(jt == 0), stop=(jt == JT - 1),
                    )
                nc.vector.tensor_mul(out=zt_sb[:, dt_i, :], in0=ps[:], in1=sbc_sb[:])

        # ---------------- phase O: out = z @ Wo + bo ----------------
        with (
            tc.tile_pool(name="po_sbuf", bufs=2) as po,
            tc.tile_pool(name="po_psum", bufs=2, space="PSUM") as pop,
        ):
            for it in range(IT):
                ps = pop.tile([P, H], mybir.dt.float32, tag="o")
                for dt_i in range(DT):
                    nc.tensor.matmul(
                        ps[:],
                        zt_sb[:, dt_i, it * P : (it + 1) * P],
                        wo_sb[:, dt_i, :],
                        start=(dt_i == 0), stop=False,
                    )
                nc.tensor.matmul(
                    ps[:], ones32_sb[0:1, :], bo_sb[0:1, :], start=False, stop=True
                )
                o_t = po.tile([P, H], f32, tag="osb")
                nc.vector.tensor_copy(out=o_t[:], in_=ps[:])
                nc.sync.dma_start(out=out[it * P : (it + 1) * P, :], in_=o_t[:])

    nc.finalize()
    return nc


def _prep_in_maps(x, Wq, bq, Wk, bk, Wv, bv, Wo, bo):
    x = np.ascontiguousarray(x, dtype=np.float32)
    shared = {
        "wq": np.ascontiguousarray(Wq, dtype=np.float32),
        "wk": np.ascontiguousarray(Wk, dtype=np.float32),
        "wv": np.ascontiguousarray(Wv, dtype=np.float32),
        "wo": np.ascontiguousarray(Wo, dtype=np.float32),
        "bqr": np.ascontiguousarray(bq, dtype=np.float32).reshape(1, D),
        "bkr": np.ascontiguousarray(bk, dtype=np.float32).reshape(1, D),
        "bvr": np.ascontiguousarray(bv, dtype=np.float32).reshape(1, D),
        "bor": np.ascontiguousarray(bo, dtype=np.float32).reshape(1, H),
        "ones_row": np.ones((1, NSH), dtype=np.float32),
        "ones_col": np.ones((P, 1), dtype=np.float32),
        "eighth_row": np.full((1, P), 0.125, dtype=np.float32),
        "ones32_row": np.ones((1, P), dtype=np.float32),
    }
    in_maps = []
    for c in range(NCORES):
        m = dict(shared)
        m["xcT"] = np.ascontiguousarray(x[c * NSH : (c + 1) * NSH, :].T)
        in_maps.append(m)
    return in_maps


def kernel(x, Wq, bq, Wk, bk, Wv, bv, Wo, bo, _want_trace=False):
    from concourse.bass_utils import run_bass_kernel_spmd

    if "nc" not in _CACHE:
        _CACHE["nc"] = _build()
    nc = _CACHE["nc"]

    in_maps = _prep_in_maps(x, Wq, bq, Wk, bk, Wv, bv, Wo, bo)
    res = run_bass_kernel_spmd(
        nc, in_maps, core_ids=list(range(NCORES)), trace=_want_trace
    )
    _CACHE["last_result"] = res
    return np.concatenate([res.results[c]["out"] for c in range(NCORES)], axis=0)


# revision 3
# speedup vs baseline: 1.3068x; 1.3068x over previous
"""Trainium2 Bass kernel for nn_MultiHeadAttention_53815940219243.

Reference computation (single-head attention with full 1024-dim contraction):
    q = x @ Wq + bq; k = x @ Wk + bk; v = x @ Wv + bv        # [4096, 1024]
    scores = softmax(q @ k.T, axis=-1) / sqrt(64)            # [4096, 4096]
    z = scores @ v                                           # [4096, 1024]
    out = z @ Wo + bo                                        # [4096, 64]

Sharding: sequence dim of Q/scores/output split across 8 cores (512 rows each).
K and V are computed sharded (each core its own 512 rows) and exchanged with
two AllGather collectives overlapped with the other projections.

Dataflow runs in "transposed score space" so no on-device transposes are
needed (see ST/UT below). High-precision matmuls use hi/lo-split bf16
operands (3 passes: hh, hl, lh), giving ~2e-4 matmul error at bf16 speed;
E=exp(S) and v are single bf16 (~1e-3), the final Wo matmul is fp32.

Softmax is computed without max subtraction: scores are ~N(0, 10.7^2), so
|S| < ~60 and exp(S) stays inside fp32/bf16 range.
"""

import numpy as np

N = 4096
D = 1024
H = 64
NCORES = 8
NSH = N // NCORES  # 512 rows per core
P = 128
DT = D // P  # 8 partition tiles over the 1024 feature dim
JT = N // P  # 32 partition tiles over the full sequence
JSH = NSH // P  # 4 sequence tiles per core shard
IT = NSH // P  # 4 output row tiles per core

_CACHE = {}


def _build():
    import concourse.mybir as mybir
    import concourse.tile as tile
    from concourse import bacc
    from contextlib import ExitStack

    f32 = mybir.dt.float32
    bf16 = mybir.dt.bfloat16

    nc = bacc.Bacc("TRN2", target_bir_lowering=False, num_devices=NCORES)

    # ---- kernel I/O (per core) ----
    xh = nc.dram_tensor("xh", [D, NSH], bf16, kind="ExternalInput")
    xl = nc.dram_tensor("xl", [D, NSH], bf16, kind="ExternalInput")
    # weights pre-arranged on host: [t, p, dt*c] so each dout-tile load is
    # one contiguous-per-partition DMA
    wqh = nc.dram_tensor("wqh", [DT, P, D], bf16, kind="ExternalInput")
    wql = nc.dram_tensor("wql", [DT, P, D], bf16, kind="ExternalInput")
    wkh = nc.dram_tensor("wkh", [DT, P, D], bf16, kind="ExternalInput")
    wkl = nc.dram_tensor("wkl", [DT, P, D], bf16, kind="ExternalInput")
    wvh = nc.dram_tensor("wvh", [2, P, DT * 512], bf16, kind="ExternalInput")
    wvl = nc.dram_tensor("wvl", [2, P, DT * 512], bf16, kind="ExternalInput")
    wo = nc.dram_tensor("wo", [D, H], f32, kind="ExternalInput")
    bq_r = nc.dram_tensor("bq_r", [1, D], bf16, kind="ExternalInput")
    bk_r = nc.dram_tensor("bk_r", [1, D], bf16, kind="ExternalInput")
    bv_r = nc.dram_tensor("bv_r", [1, D], bf16, kind="ExternalInput")
    bo_r = nc.dram_tensor("bo_r", [1, H], f32, kind="ExternalInput")
    ones_row = nc.dram_tensor("ones_row", [1, NSH], bf16, kind="ExternalInput")
    ones_col = nc.dram_tensor("ones_col", [P, 1], bf16, kind="ExternalInput")
    eighth_row = nc.dram_tensor("eighth_row", [1, P], f32, kind="ExternalInput")
    ones32_row = nc.dram_tensor("ones32_row", [1, P], f32, kind="ExternalInput")

    out = nc.dram_tensor("out", [NSH, H], f32, kind="ExternalOutput")

    # ---- internal DRAM for the collectives ----
    ag_in_kt = nc.dram_tensor("ag_in_kt", [2, D, NSH], bf16)
    ag_out_kt = nc.dram_tensor("ag_out_kt", [NCORES, 2, D, NSH], bf16, addr_space="Shared")
    ag_in_v = nc.dram_tensor("ag_in_v", [NSH, D], bf16)
    ag_out_v = nc.dram_tensor("ag_out_v", [NCORES, NSH, D], bf16, addr_space="Shared")

    with tile.TileContext(nc) as tc, ExitStack() as ctx:
        persist = ctx.enter_context(tc.tile_pool(name="persist", bufs=1))
        small = ctx.enter_context(tc.tile_pool(name="small", bufs=1))

        qth_sb = persist.tile([P, DT, NSH], bf16, tag="qth")
        qtl_sb = persist.tile([P, DT, NSH], bf16, tag="qtl")
        et_sb = persist.tile([P, JT, NSH], bf16, tag="et")        # 32KB/part
        vf_sb = persist.tile([P, JT, D], bf16, tag="vf")          # 64KB/part
        zt_sb = persist.tile([P, DT, NSH], f32, tag="zt")         # 16KB/part
        sbc_sb = persist.tile([P, NSH], f32, tag="sbc")

        ones_row_sb = small.tile([1, NSH], bf16, tag="onesr")
        ones_col_sb = small.tile([P, 1], bf16, tag="onesc")
        eighth_sb = small.tile([1, P], f32, tag="eighth")
        ones32_sb = small.tile([1, P], f32, tag="ones32")
        bq_sb = small.tile([1, D], bf16, tag="bq")
        bk_sb = small.tile([1, D], bf16, tag="bk")
        bv_sb = small.tile([1, D], bf16, tag="bv")
        bo_sb = small.tile([1, H], f32, tag="bo")
        wo_sb = small.tile([P, DT, H], f32, tag="wo")
        den_row = small.tile([1, NSH], f32, tag="denr")

        nc.sync.dma_start(out=ones_row_sb[:], in_=ones_row[:, :])
        nc.sync.dma_start(out=ones_col_sb[:], in_=ones_col[:, :])
        nc.sync.dma_start(out=eighth_sb[:], in_=eighth_row[:, :])
        nc.sync.dma_start(out=ones32_sb[:], in_=ones32_row[:, :])
        nc.sync.dma_start(out=bq_sb[:], in_=bq_r[:, :])
        nc.sync.dma_start(out=bk_sb[:], in_=bk_r[:, :])
        nc.sync.dma_start(out=bv_sb[:], in_=bv_r[:, :])
        nc.sync.dma_start(out=bo_sb[:], in_=bo_r[:, :])
        nc.sync.dma_start(out=wo_sb[:], in_=wo[:, :].rearrange("(t p) h -> p t h", p=P))

        # ---------------- phase A: projections ----------------
        with (
            tc.tile_pool(name="pa_x", bufs=1) as pax,
            tc.tile_pool(name="pa_sbuf", bufs=3) as pa,
            tc.tile_pool(name="pa_w", bufs=2) as paw,
            tc.tile_pool(name="pa_psum", bufs=3, space="PSUM") as pap,
        ):
            xh_sb = pax.tile([P, DT, NSH], bf16, tag="xh")
            xl_sb = pax.tile([P, DT, NSH], bf16, tag="xl")
            nc.sync.dma_start(out=xh_sb[:], in_=xh[:, :].rearrange("(t p) i -> p t i", p=P))
            nc.sync.dma_start(out=xl_sb[:], in_=xl[:, :].rearrange("(t p) i -> p t i", p=P))

            def proj_qk(w_h, w_l, b_sb, t):
                """One dout-tile of a hi/lo-split projection W^T @ xT + b."""
                wt_h = paw.tile([P, DT, P], bf16, tag="wqkh")
                wt_l = paw.tile([P, DT, P], bf16, tag="wqkl")
                nc.sync.dma_start(out=wt_h[:], in_=w_h[t].rearrange("p (dt c) -> p dt c", c=P))
                nc.sync.dma_start(out=wt_l[:], in_=w_l[t].rearrange("p (dt c) -> p dt c", c=P))
                ps = pap.tile([P, NSH], mybir.dt.float32, tag="pa")
                for dt_i in range(DT):
                    nc.tensor.matmul(ps[:], wt_h[:, dt_i, :], xh_sb[:, dt_i, :],
                                     start=(dt_i == 0), stop=False)
                    nc.tensor.matmul(ps[:], wt_h[:, dt_i, :], xl_sb[:, dt_i, :],
                                     start=False, stop=False)
                    nc.tensor.matmul(ps[:], wt_l[:, dt_i, :], xh_sb[:, dt_i, :],
                                     start=False, stop=False)
                nc.tensor.matmul(ps[:], b_sb[0:1, t * P : (t + 1) * P],
                                 ones_row_sb[0:1, :], start=False, stop=True)
                return ps

            # K^T shard -> split hi/lo -> ag_in_kt
            for t in range(DT):
                ps = proj_qk(wkh, wkl, bk_sb, t)
                kt_h = pa.tile([P, NSH], bf16, tag="kth")
                kt_l = pa.tile([P, NSH], bf16, tag="ktl")
                nc.vector.tensor_copy(out=kt_h[:], in_=ps[:])
                nc.vector.tensor_sub(out=kt_l[:], in0=ps[:], in1=kt_h[:])
                nc.sync.dma_start(out=ag_in_kt[0, t * P : (t + 1) * P, :], in_=kt_h[:])
                nc.sync.dma_start(out=ag_in_kt[1, t * P : (t + 1) * P, :], in_=kt_l[:])

            nc.gpsimd.collective_compute(
                "AllGather", mybir.AluOpType.bypass,
                replica_groups=[list(range(NCORES))],
                ins=[ag_in_kt[:, :, :].opt()],
                outs=[ag_out_kt[:, :, :, :].opt()],
            )

            # v shard (natural layout, single bf16) -> ag_in_v
            for b in range(2):
                wv_h = paw.tile([P, DT, 512], bf16, tag="wvh")
                wv_l = paw.tile([P, DT, 512], bf16, tag="wvl")
                nc.sync.dma_start(out=wv_h[:], in_=wvh[b].rearrange("p (dt c) -> p dt c", c=512))
                nc.sync.dma_start(out=wv_l[:], in_=wvl[b].rearrange("p (dt c) -> p dt c", c=512))
                for j in range(JSH):
                    ps = pap.tile([P, 512], mybir.dt.float32, tag="pa")
                    for dt_i in range(DT):
                        nc.tensor.matmul(ps[:], xh_sb[:, dt_i, j * P : (j + 1) * P],
                                         wv_h[:, dt_i, :], start=(dt_i == 0), stop=False)
                        nc.tensor.matmul(ps[:], xh_sb[:, dt_i, j * P : (j + 1) * P],
                                         wv_l[:, dt_i, :], start=False, stop=False)
                        nc.tensor.matmul(ps[:], xl_sb[:, dt_i, j * P : (j + 1) * P],
                                         wv_h[:, dt_i, :], start=False, stop=False)
                    nc.tensor.matmul(ps[:], ones_row_sb[0:1, 0:P],
                                     bv_sb[0:1, b * 512 : (b + 1) * 512],
                                     start=False, stop=True)
                    v_t = pa.tile([P, 512], bf16, tag="vsh")
                    nc.vector.tensor_copy(out=v_t[:], in_=ps[:])
                    nc.sync.dma_start(
                        out=ag_in_v[j * P : (j + 1) * P, b * 512 : (b + 1) * 512],
                        in_=v_t[:],
                    )

            nc.gpsimd.collective_compute(
                "AllGather", mybir.AluOpType.bypass,
                replica_groups=[list(range(NCORES))],
                ins=[ag_in_v[:, :].opt()],
                outs=[ag_out_v[:, :, :].opt()],
            )

            # Q^T (resident, split hi/lo)
            for t in range(DT):
                ps = proj_qk(wqh, wql, bq_sb, t)
                nc.vector.tensor_copy(out=qth_sb[:, t, :], in_=ps[:])
                nc.vector.tensor_sub(out=qtl_sb[:, t, :], in0=ps[:], in1=qth_sb[:, t, :])

        # full v load (overlaps the S phase below)
        nc.sync.dma_start(
            out=vf_sb[:], in_=ag_out_v[:, :, :].rearrange("r (q p) d -> p (r q) d", p=P)
        )

        # ---------------- phase S: scores + exp + denominator ----------------
        with (
            tc.tile_pool(name="ps_kt", bufs=2) as pskt,
            tc.tile_pool(name="ps_psum", bufs=3, space="PSUM") as psp,
            tc.tile_pool(name="ps_den", bufs=1, space="PSUM") as psd,
        ):
            den_ps = psd.tile([1, NSH], mybir.dt.float32, tag="den")
            for r in range(NCORES):
                kt_r = pskt.tile([P, 2, DT, NSH], bf16, tag="ktr")
                nc.sync.dma_start(
                    out=kt_r[:],
                    in_=ag_out_kt[r, :, :, :].rearrange("s (t p) j -> p s t j", p=P),
                )
                for jj in range(JSH):
                    jt = r * JSH + jj
                    ps = psp.tile([P, NSH], mybir.dt.float32, tag="st")
                    jsl = slice(jj * P, (jj + 1) * P)
                    for dt_i in range(DT):
                        last = dt_i == DT - 1
                        nc.tensor.matmul(ps[:], kt_r[:, 0, dt_i, jsl], qth_sb[:, dt_i, :],
                                         start=(dt_i == 0), stop=False)
                        nc.tensor.matmul(ps[:], kt_r[:, 0, dt_i, jsl], qtl_sb[:, dt_i, :],
                                         start=False, stop=False)
                        nc.tensor.matmul(ps[:], kt_r[:, 1, dt_i, jsl], qth_sb[:, dt_i, :],
                                         start=False, stop=last)
                    nc.scalar.activation(out=et_sb[:, jt, :], in_=ps[:],
                                         func=mybir.ActivationFunctionType.Exp)
                    nc.tensor.matmul(den_ps[:], ones_col_sb[:, 0:1], et_sb[:, jt, :],
                                     start=(jt == 0), stop=(jt == JT - 1))

            # s = 1/(8*den) broadcast to 128 partitions (fp32 matmul)
            nc.vector.reciprocal(out=den_row[:], in_=den_ps[:])
            bc_ps = psp.tile([P, NSH], mybir.dt.float32, tag="bc")
            nc.tensor.matmul(bc_ps[:], eighth_sb[0:1, :], den_row[0:1, :],
                             start=True, stop=True)
            nc.vector.tensor_copy(out=sbc_sb[:], in_=bc_ps[:])

        # ---------------- phase U: z^T ----------------
        with tc.tile_pool(name="pu_psum", bufs=2, space="PSUM") as pup:
            for dt_i in range(DT):
                ps = pup.tile([P, NSH], mybir.dt.float32, tag="ut")
                dsl = slice(dt_i * P, (dt_i + 1) * P)
                for jt in range(JT):
                    nc.tensor.matmul(ps[:], vf_sb[:, jt, dsl], et_sb[:, jt, :],
                                     start=(jt == 0), stop=(jt == JT - 1))
                nc.vector.tensor_mul(out=zt_sb[:, dt_i, :], in0=ps[:], in1=sbc_sb[:])

        # ---------------- phase O: out = z @ Wo + bo (fp32) ----------------
        with (
            tc.tile_pool(name="po_sbuf", bufs=2) as po,
            tc.tile_pool(name="po_psum", bufs=2, space="PSUM") as pop,
        ):
            for it in range(IT):
                ps = pop.tile([P, H], mybir.dt.float32, tag="o")
                isl = slice(it * P, (it + 1) * P)
                for dt_i in range(DT):
                    nc.tensor.matmul(ps[:], zt_sb[:, dt_i, isl], wo_sb[:, dt_i, :],
                                     start=(dt_i == 0), stop=False)
                nc.tensor.matmul(ps[:], ones32_sb[0:1, :], bo_sb[0:1, :],
                                 start=False, stop=True)
                o_t = po.tile([P, H], f32, tag="osb")
                nc.vector.tensor_copy(out=o_t[:], in_=ps[:])
                nc.sync.dma_start(out=out[isl, :], in_=o_t[:])

    nc.finalize()
    return nc


def _split_bf16(a):
    import ml_dtypes
    a = np.ascontiguousarray(a, dtype=np.float32)
    hi = a.astype(ml_dtypes.bfloat16)
    lo = (a - hi.astype(np.float32)).astype(ml_dtypes.bfloat16)
    return hi, lo


def _prep_in_maps(x, Wq, bq, Wk, bk, Wv, bv, Wo, bo):
    import ml_dtypes
    bf = ml_dtypes.bfloat16
    x = np.ascontiguousarray(x, dtype=np.float32)

    def arr_qk(W):
        h, l = _split_bf16(W)
        # [din, dout] -> [t, p, dt*c]: W'[t, p, dt, c] = W[dt*128+p, t*128+c]
        def re(a):
            return np.ascontiguousarray(
                a.reshape(DT, P, DT, P).transpose(2, 1, 0, 3).reshape(DT, P, D)
            )
        return re(h), re(l)

    def arr_v(W):
        h, l = _split_bf16(W)
        def re(a):
            return np.ascontiguousarray(
                a.reshape(DT, P, 2, 512).transpose(2, 1, 0, 3).reshape(2, P, DT * 512)
            )
        return re(h), re(l)

    wqh, wql = arr_qk(Wq)
    wkh, wkl = arr_qk(Wk)
    wvh, wvl = arr_v(Wv)

    shared = {
        "wqh": wqh, "wql": wql, "wkh": wkh, "wkl": wkl, "wvh": wvh, "wvl": wvl,
        "wo": np.ascontiguousarray(Wo, dtype=np.float32),
        "bq_r": np.asarray(bq, np.float32).reshape(1, D).astype(bf),
        "bk_r": np.asarray(bk, np.float32).reshape(1, D).astype(bf),
        "bv_r": np.asarray(bv, np.float32).reshape(1, D).astype(bf),
        "bo_r": np.ascontiguousarray(bo, dtype=np.float32).reshape(1, H),
        "ones_row": np.ones((1, NSH), dtype=bf),
        "ones_col": np.ones((P, 1), dtype=bf),
        "eighth_row": np.full((1, P), 0.125, dtype=np.float32),
        "ones32_row": np.ones((1, P), dtype=np.float32),
    }
    in_maps = []
    for c in range(NCORES):
        xcT = np.ascontiguousarray(x[c * NSH : (c + 1) * NSH, :].T)
        xch, xcl = _split_bf16(xcT)
        m = dict(shared)
        m["xh"] = xch
        m["xl"] = xcl
        in_maps.append(m)
    return in_maps


def kernel(x, Wq, bq, Wk, bk, Wv, bv, Wo, bo):
    from concourse.bass_utils import run_bass_kernel_spmd

    if "nc" not in _CACHE:
        _CACHE["nc"] = _build()
    nc = _CACHE["nc"]

    in_maps = _prep_in_maps(x, Wq, bq, Wk, bk, Wv, bv, Wo, bo)
    res = run_bass_kernel_spmd(nc, in_maps, core_ids=list(range(NCORES)))
    _CACHE["last_result"] = res
    return np.concatenate([res.results[c]["out"] for c in range(NCORES)], axis=0)
